# revision 2
# baseline (speedup 1.0000x reference)
"""Trainium2 Bass kernel v2: out = e + e @ B @ A^T  (low-rank residual update).

e: [4, 4096, 4096] f32, A/B: [4096, 16] f32.
Data-parallel over rows (16384 rows -> 2048 rows/core on 8 cores).

v2 changes vs baseline:
  - st_rows=256 supertiles with a deep e-buffer pipeline so the (serialized,
    ~360 GB/s) DMA path never waits on compute or buffer reuse.
  - B / A^T loaded as f32 via HWDGE (nc.sync) before the first e load into
    small staging tiles, then rounded on-chip to float32r with DVE/ACT
    copies (the SWDGE cast path made them queue behind the first two 8 MiB
    e loads -> ~25 us of dead startup time; a raw bitcast is rejected by
    the BIR verifier because f32r inputs must come from rounding producers).
  - e stores issued from gpsimd (SWDGE) so the SP queue only carries loads:
    a store waiting on compute can never head-of-line-block the next load.
  - Software-pipelined emission: phase A (transpose+copy+mm1) of supertile
    n+1 is emitted BEFORE phase B (mm2+add) of supertile n, so the FIFO
    engine queues let tile n+1's front end overlap tile n's back end.
  - PSUM->SBUF transpose copies mostly/entirely on ACT; DVE keeps the adds.
"""

import sys

sys.path.insert(0, "/opt/trn_rl_repo")

import numpy as np

import concourse.bass as bass
import concourse.mybir as mybir
import concourse.tile as tile
from concourse.masks import make_identity


def _split_waits(nc, max_w=1):
    """The walrus in this container rejects instructions carrying more than
    ~2 sync-waits. Hoist extra waits onto same-engine NOPs placed directly
    before the offending instruction (engines execute their stream in
    order, so this is semantics-preserving)."""
    for f in nc.m.functions:
        for blk in f.blocks:
            insts = blk.instructions
            out = []
            changed = False
            for inst in insts:
                si = inst.sync_info
                if si is not None and si.on_wait and len(si.on_wait) > max_w:
                    waits = list(si.on_wait)
                    for j, w in enumerate(waits[max_w:]):
                        out.append(
                            mybir.InstNoOp(
                                name=f"{inst.name}-wsplit{j}",
                                sync_info=mybir.SyncInfo(on_wait=[w], on_update=[]),
                                bass_nofuse=True,
                                engine=inst.engine,
                            )
                        )
                    si.on_wait = waits[:max_w]
                    changed = True
                out.append(inst)
            if changed:
                blk.instructions = out


DIM = 4096
RANK = 16
N_CORES = 8
ROWS_TOTAL = 4 * 4096
ROWS_PER_CORE = ROWS_TOTAL // N_CORES  # 2048

# float32r streams at full PE rate for moving free-dim >= 256.
MM_DT = mybir.dt.float32r


def build_nc(
    rows_per_core=ROWS_PER_CORE,
    st_rows=256,
    mm_dt=MM_DT,
    ebufs=5,
    dve_copy_every=0,       # 0 = all PSUM->SBUF copies on ACT; k>0: every k-th on DVE
    store_on_gpsimd=True,
    swp=True,               # software-pipeline: phase A of st+1 before phase B of st
    etbufs=4,
    ttbufs=3,
    repeat=1,               # re-run the whole pass N times (timing harness only)
    split_waits=True,
):
    assert st_rows % 128 == 0 and rows_per_core % st_rows == 0
    rb = st_rows // 128          # row blocks per supertile
    n_st = rows_per_core // st_rows
    kc = DIM // 128              # contraction chunks
    nch = DIM // 512             # output column chunks

    nc = bass.Bass("TRN2", target_bir_lowering=False, debug=False)
    e_in = nc.dram_tensor(
        "e_in", [rows_per_core, DIM], mybir.dt.float32, kind="ExternalInput"
    )
    b_in = nc.dram_tensor("b_in", [DIM, RANK], mybir.dt.float32, kind="ExternalInput")
    at_in = nc.dram_tensor(
        "at_in", [RANK, DIM], mybir.dt.float32, kind="ExternalInput"
    )
    out_d = nc.dram_tensor(
        "out_d", [rows_per_core, DIM], mybir.dt.float32, kind="ExternalOutput"
    )

    with tile.TileContext(nc) as tc:
        with (
            tc.tile_pool(name="const", bufs=1) as cpool,
            tc.tile_pool(name="stage", bufs=1) as stage,
            tc.tile_pool(name="epool", bufs=ebufs) as epool,
            tc.tile_pool(name="etpool", bufs=etbufs) as etpool,
            tc.tile_pool(name="ttpool", bufs=ttbufs) as ttpool,
            tc.tile_pool(name="pstr", bufs=2, space="PSUM") as pstr,
            tc.tile_pool(name="pst", bufs=2, space="PSUM") as pst,
            tc.tile_pool(name="psy", bufs=4, space="PSUM") as psy,
        ):
            # B rearranged so chunk k is b_r[:, k*16:(k+1)*16] = B[k*128:(k+1)*128, :]
            # Loaded as plain f32 on the HWDGE (sync) queue ahead of the e loads
            # (tiny), then rounded on-chip into float32r tiles: f32r matmul
            # operands must be produced by a rounding op (BIR verifier rule),
            # and the SWDGE cast-during-DMA path would queue behind e loads.
            b_f32 = stage.tile([128, kc * RANK], mybir.dt.float32, name="b_f32")
            nc.sync.dma_start(
                out=b_f32.rearrange("p (k j) -> p k j", j=RANK),
                in_=b_in.ap().rearrange("(k p) j -> p k j", p=128),
            )
            b_r = cpool.tile([128, kc * RANK], mm_dt, name="b_r")
            nc.vector.tensor_copy(out=b_r, in_=b_f32)

            at_r = cpool.tile([RANK, DIM], mm_dt, name="at_r")
            for h in range(2):
                at_f32 = stage.tile([RANK, DIM // 2], mybir.dt.float32, name="at_f32")
                nc.sync.dma_start(
                    out=at_f32, in_=at_in.ap()[:, h * (DIM // 2) : (h + 1) * (DIM // 2)]
                )
                nc.scalar.copy(at_r[:, h * (DIM // 2) : (h + 1) * (DIM // 2)], at_f32)

            ident = cpool.tile([128, 128], mybir.dt.float32, name="ident")
            make_identity(nc, ident)

            e_ap = e_in.ap()
            o_ap = out_d.ap()
            store_eng = nc.gpsimd if store_on_gpsimd else nc.sync

            ets_tiles = {}  # st -> et tile
            tts_tiles = {}  # st -> tts tile

            def emit_load(key):
                r0 = key[1] * st_rows
                et = epool.tile([128, rb * DIM], mybir.dt.float32, name="et")
                nc.sync.dma_start(
                    out=et.rearrange("p (b c) -> p b c", c=DIM),
                    in_=e_ap[r0 : r0 + st_rows, :].rearrange(
                        "(b p) c -> p b c", p=128
                    ),
                )
                ets_tiles[key] = et

            def emit_phase_a(key):
                et = ets_tiles[key]
                tps = pst.tile([RANK, st_rows], mybir.dt.float32, name="tps")
                for k in range(kc):
                    trp = pstr.tile([128, st_rows], mybir.dt.float32, name="trp")
                    for b in range(rb):
                        nc.tensor.transpose(
                            trp[:, b * 128 : (b + 1) * 128],
                            et[:, b * DIM + k * 128 : b * DIM + (k + 1) * 128],
                            ident,
                        )
                    ets = etpool.tile([128, st_rows], mm_dt, name="ets")
                    if dve_copy_every and k % dve_copy_every == 0:
                        nc.vector.tensor_copy(out=ets, in_=trp)
                    else:
                        nc.scalar.copy(ets, trp)
                    nc.tensor.matmul(
                        tps,
                        b_r[:, k * RANK : (k + 1) * RANK],
                        ets,
                        start=(k == 0),
                        stop=(k == kc - 1),
                    )
                tts = ttpool.tile([RANK, st_rows], mm_dt, name="tts")
                nc.vector.tensor_copy(out=tts, in_=tps)
                tts_tiles[key] = tts

            def emit_phase_b(key):
                et = ets_tiles[key]
                tts = tts_tiles.pop(key)
                r0 = key[1] * st_rows
                for b in range(rb):
                    for n in range(nch):
                        yp = psy.tile([128, 512], mybir.dt.float32, name="yp")
                        nc.tensor.matmul(
                            yp,
                            tts[:, b * 128 : (b + 1) * 128],
                            at_r[:, n * 512 : (n + 1) * 512],
                            start=True,
                            stop=True,
                        )
                        seg = et[:, b * DIM + n * 512 : b * DIM + (n + 1) * 512]
                        nc.vector.tensor_add(out=seg, in0=seg, in1=yp)
                store_eng.dma_start(
                    out=o_ap[r0 : r0 + st_rows, :].rearrange(
                        "(b p) c -> p b c", p=128
                    ),
                    in_=et.rearrange("p (b c) -> p b c", c=DIM),
                )
                ets_tiles.pop(key)

            # Flat (rep, st) sequence so the pipeline stays full across
            # repeats (repeat>1 is only used for hardware timing).
            seq = [(r, s) for r in range(repeat) for s in range(n_st)]
            if swp:
                emit_load(seq[0])
                emit_phase_a(seq[0])
                for i in range(1, len(seq)):
                    emit_load(seq[i])
                    emit_phase_a(seq[i])
                    emit_phase_b(seq[i - 1])
                emit_phase_b(seq[-1])
            else:
                for key in seq:
                    emit_load(key)
                    emit_phase_a(key)
                    emit_phase_b(key)
    if split_waits:
        _split_waits(nc)
    return nc


_NC_CACHE = {}


def _get_nc(rows_per_core=ROWS_PER_CORE):
    key = rows_per_core
    if key not in _NC_CACHE:
        _NC_CACHE[key] = build_nc(rows_per_core)
    return _NC_CACHE[key]


def kernel(e, A, B):
    from concourse.bass_utils import run_bass_kernel_spmd

    e = np.asarray(e, dtype=np.float32)
    A = np.asarray(A, dtype=np.float32)
    B = np.asarray(B, dtype=np.float32)
    batch, seq, dim = e.shape
    rows = batch * seq
    e2 = np.ascontiguousarray(e.reshape(rows, dim))
    at = np.ascontiguousarray(A.T)

    rpc = rows // N_CORES
    in_maps = [
        {
            "e_in": np.ascontiguousarray(e2[i * rpc : (i + 1) * rpc]),
            "b_in": B,
            "at_in": at,
        }
        for i in range(N_CORES)
    ]
    nc = _get_nc(rpc)
    res = run_bass_kernel_spmd(nc, in_maps, core_ids=list(range(N_CORES)))
    out = np.concatenate([res.results[i]["out_d"] for i in range(N_CORES)], axis=0)
    return out.reshape(batch, seq, dim).astype(np.float32)


# revision 3
# speedup vs baseline: 1.3312x; 1.3312x over previous
"""Trainium2 Bass kernel v3: out = e + e @ B @ A^T  (low-rank residual update).

e: [4, 4096, 4096] f32, A/B: [4096, 16] f32.
Data-parallel over rows (16384 rows -> 2048 rows/core on 8 cores).

v3 changes vs v2:
  - e is cast to float16 HOST-side and shipped to device DRAM as f16: the
    kernel's HBM read traffic halves (32 -> 16 MiB/core). Stores stay f32
    (dtype contract), upcast during the SWDGE store DMA. Error budget: f16
    keeps ~5e-4 relative precision and the low-rank correction is only a few
    percent of e, so total rel err ~1e-3 vs the 2e-2 gate.
  - B is pre-rearranged and cast host-side to b2[p, k*16+j] = B[k*128+p, j]
    (f16 [128, 512]); A^T cast to f16 [16, 4096]. Both load contiguous at
    full DMA rate with no on-chip staging/rounding (f16 matmuls replace the
    f32r path, same 1 cycle/row PE rate, and f16 transposes are 2x cheaper
    than f32).
  - st_rows=256 supertiles, 6-deep f16 e-buffer pipeline (f16 tiles halve
    SBUF), loads on SP / stores on gpsimd, software-pipelined emission as
    in v2.
"""

import sys

sys.path.insert(0, "/opt/trn_rl_repo")

import numpy as np

import concourse.bass as bass
import concourse.mybir as mybir
import concourse.tile as tile
from concourse.masks import make_identity


def _split_waits(nc, max_w=1):
    """The walrus in this container rejects instructions carrying more than
    ~2 sync-waits. Hoist extra waits onto same-engine NOPs placed directly
    before the offending instruction (engines execute their stream in
    order, so this is semantics-preserving)."""
    for f in nc.m.functions:
        for blk in f.blocks:
            insts = blk.instructions
            out = []
            changed = False
            for inst in insts:
                si = inst.sync_info
                if si is not None and si.on_wait and len(si.on_wait) > max_w:
                    waits = list(si.on_wait)
                    for j, w in enumerate(waits[max_w:]):
                        out.append(
                            mybir.InstNoOp(
                                name=f"{inst.name}-wsplit{j}",
                                sync_info=mybir.SyncInfo(on_wait=[w], on_update=[]),
                                bass_nofuse=True,
                                engine=inst.engine,
                            )
                        )
                    si.on_wait = waits[:max_w]
                    changed = True
                out.append(inst)
            if changed:
                blk.instructions = out


DIM = 4096
RANK = 16
N_CORES = 8
ROWS_TOTAL = 4 * 4096
ROWS_PER_CORE = ROWS_TOTAL // N_CORES  # 2048

E_DT = mybir.dt.float16      # on-device e / matmul operand dtype
E_NP = np.float16


def build_nc(
    rows_per_core=ROWS_PER_CORE,
    st_rows=256,
    ebufs=6,
    dve_copy_every=0,       # 0 = all phase-A PSUM->SBUF copies on ACT; k>0: every k-th on DVE
    store_on_gpsimd=True,
    swp=True,               # software-pipeline: phase A of st+1 before phase B of st
    identity_accum=False,   # add e via PE identity-matmul accumulation; PSUM->SBUF
                            # output moves become copies split across DVE/ACT
    interleave=True,        # emit phase-B chunks of st between k-chunks of st+1
    preload_all=False,      # emit every load before any store; stores then go on
                            # the SP queue (no FIFO hazard) and gpsimd is freed
                            # to take a share of the adds as a third mover
    pool_adds=0,            # out-chunks per supertile whose add runs on gpsimd
    sep_out=True,           # adds write a separate f32 out tile (et stays f16
                            # read-only; stores become plain f32 DMAs)
    pair_k=True,            # process k-chunks in pairs sharing one PSUM transpose
                            # tile -> one [128, 2*st_rows] copy per 2 chunks
    obufs=3,
    etbufs=4,
    ttbufs=3,
    repeat=1,               # re-run the whole pass N times (timing harness only)
    split_waits=True,
):
    assert st_rows % 128 == 0 and rows_per_core % st_rows == 0
    rb = st_rows // 128          # row blocks per supertile
    n_st = rows_per_core // st_rows
    kc = DIM // 128              # contraction chunks
    nch = DIM // 512             # output column chunks

    nc = bass.Bass("TRN2", target_bir_lowering=False, debug=False)
    e_in = nc.dram_tensor("e_in", [rows_per_core, DIM], E_DT, kind="ExternalInput")
    # b2[p, k*16+j] = B[k*128+p, j], host-prepared (contiguous full-rate load)
    b_in = nc.dram_tensor("b_in", [128, kc * RANK], E_DT, kind="ExternalInput")
    at_in = nc.dram_tensor("at_in", [RANK, DIM], E_DT, kind="ExternalInput")
    out_d = nc.dram_tensor(
        "out_d", [rows_per_core, DIM], mybir.dt.float32, kind="ExternalOutput"
    )

    with tile.TileContext(nc) as tc:
        with (
            tc.tile_pool(name="const", bufs=1) as cpool,
            tc.tile_pool(name="epool", bufs=ebufs) as epool,
            tc.tile_pool(name="opool", bufs=obufs) as opool,
            tc.tile_pool(name="etpool", bufs=etbufs) as etpool,
            tc.tile_pool(name="ttpool", bufs=ttbufs) as ttpool,
            tc.tile_pool(name="pstr", bufs=2, space="PSUM") as pstr,
            tc.tile_pool(name="pst", bufs=2, space="PSUM") as pst,
            tc.tile_pool(name="psy", bufs=4, space="PSUM") as psy,
        ):
            b_r = cpool.tile([128, kc * RANK], E_DT, name="b_r")
            nc.sync.dma_start(out=b_r, in_=b_in.ap()[:, :])
            at_r = cpool.tile([RANK, DIM], E_DT, name="at_r")
            nc.sync.dma_start(out=at_r, in_=at_in.ap()[:, :])

            ident = cpool.tile([128, 128], E_DT, name="ident")
            make_identity(nc, ident)

            e_ap = e_in.ap()
            o_ap = out_d.ap()
            if preload_all:
                store_eng = nc.sync
            else:
                store_eng = nc.gpsimd if store_on_gpsimd else nc.sync
            # evenly-spread subset of out-chunk indices whose add runs on Pool
            n_bch_total = (st_rows // 128) * (DIM // 512)
            pool_set = {
                i
                for i in range(n_bch_total)
                if (i * pool_adds) // n_bch_total != ((i + 1) * pool_adds) // n_bch_total
            }

            ets_tiles = {}  # key -> et tile
            tts_tiles = {}  # key -> tts tile
            out_tiles = {}  # key -> f32 out tile (sep_out)

            def emit_load(key):
                r0 = key[1] * st_rows
                et = epool.tile([128, rb * DIM], E_DT, name="et")
                nc.sync.dma_start(
                    out=et.rearrange("p (b c) -> p b c", c=DIM),
                    in_=e_ap[r0 : r0 + st_rows, :].rearrange(
                        "(b p) c -> p b c", p=128
                    ),
                )
                ets_tiles[key] = et

            bch = [(b, n) for b in range(rb) for n in range(nch)]
            n_bch = len(bch)                      # 16 output chunks per supertile
            every = max(1, kc // n_bch)           # k-chunks per interleaved B chunk
            bcnt = {}                             # key -> output chunks emitted so far

            def emit_b_chunk(key, i):
                """Emit output chunk i (mm2 [+ identity-accum] + PSUM->SBUF move)
                of supertile `key`."""
                et = ets_tiles[key]
                tts = tts_tiles[key]
                b, n = bch[i]
                seg = et[:, b * DIM + n * 512 : b * DIM + (n + 1) * 512]
                if sep_out:
                    if key not in out_tiles:
                        out_tiles[key] = opool.tile(
                            [128, rb * DIM], mybir.dt.float32, name="ot"
                        )
                    ot = out_tiles[key]
                    dst = ot[:, b * DIM + n * 512 : b * DIM + (n + 1) * 512]
                else:
                    dst = seg
                yp = psy.tile([128, 512], mybir.dt.float32, name="yp")
                nc.tensor.matmul(
                    yp,
                    tts[:, b * 128 : (b + 1) * 128],
                    at_r[:, n * 512 : (n + 1) * 512],
                    start=True,
                    stop=not identity_accum,
                )
                if identity_accum:
                    nc.tensor.matmul(yp, ident, seg, start=False, stop=True)
                    if i % 2 == 0:
                        nc.vector.tensor_copy(out=dst, in_=yp)
                    else:
                        nc.scalar.copy(dst, yp)
                elif i in pool_set:
                    nc.gpsimd.tensor_add(out=dst, in0=seg, in1=yp)
                else:
                    nc.vector.tensor_add(out=dst, in0=seg, in1=yp)

            def emit_finish(key):
                """Emit any remaining output chunks + the store of `key`."""
                for i in range(bcnt.get(key, 0), n_bch):
                    emit_b_chunk(key, i)
                bcnt.pop(key, None)
                et = ets_tiles.pop(key)
                tts_tiles.pop(key)
                src = out_tiles.pop(key) if sep_out else et
                r0 = key[1] * st_rows
                store_eng.dma_start(
                    out=o_ap[r0 : r0 + st_rows, :].rearrange(
                        "(b p) c -> p b c", p=128
                    ),
                    in_=src.rearrange("p (b c) -> p b c", c=DIM),
                )

            def emit_phase_a(key, prev=None):
                """Transpose+copy+mm1 for `key`; when `prev` is given and
                interleaving is on, spread prev's output chunks between the
                k-chunks so no engine queue serializes whole phases."""
                et = ets_tiles[key]
                tps = pst.tile([RANK, st_rows], mybir.dt.float32, name="tps")
                grp = 2 if pair_k else 1
                n_units = kc // grp
                ev = max(1, n_units // n_bch)
                for kk in range(n_units):
                    trp = pstr.tile([128, grp * st_rows], E_DT, name="trp")
                    ets = etpool.tile([128, grp * st_rows], E_DT, name="ets")
                    for j in range(grp):
                        k = kk * grp + j
                        for b in range(rb):
                            nc.tensor.transpose(
                                trp[:, j * st_rows + b * 128 : j * st_rows + (b + 1) * 128],
                                et[:, b * DIM + k * 128 : b * DIM + (k + 1) * 128],
                                ident,
                            )
                    if dve_copy_every and kk % dve_copy_every == 0:
                        nc.vector.tensor_copy(out=ets, in_=trp)
                    else:
                        nc.scalar.copy(ets, trp)
                    for j in range(grp):
                        k = kk * grp + j
                        nc.tensor.matmul(
                            tps,
                            b_r[:, k * RANK : (k + 1) * RANK],
                            ets[:, j * st_rows : (j + 1) * st_rows],
                            start=(k == 0),
                            stop=(k == kc - 1),
                        )
                    if prev is not None and interleave and kk % ev == ev - 1:
                        i = bcnt.get(prev, 0)
                        if i < n_bch:
                            emit_b_chunk(prev, i)
                            bcnt[prev] = i + 1
                tts = ttpool.tile([RANK, st_rows], E_DT, name="tts")
                nc.vector.tensor_copy(out=tts, in_=tps)
                tts_tiles[key] = tts

            seq = [(r, s) for r in range(repeat) for s in range(n_st)]
            if preload_all:
                assert ebufs >= n_st * repeat, "preload_all needs a buffer per tile"
                for key in seq:
                    emit_load(key)
                emit_phase_a(seq[0])
                for i in range(1, len(seq)):
                    emit_phase_a(seq[i], prev=seq[i - 1])
                    emit_finish(seq[i - 1])
                emit_finish(seq[-1])
            elif swp:
                emit_load(seq[0])
                emit_phase_a(seq[0])
                for i in range(1, len(seq)):
                    emit_load(seq[i])
                    emit_phase_a(seq[i], prev=seq[i - 1])
                    emit_finish(seq[i - 1])
                emit_finish(seq[-1])
            else:
                for key in seq:
                    emit_load(key)
                    emit_phase_a(key)
                    emit_finish(key)
    if split_waits:
        _split_waits(nc)
    return nc


_NC_CACHE = {}


def _get_nc(rows_per_core=ROWS_PER_CORE):
    key = rows_per_core
    if key not in _NC_CACHE:
        _NC_CACHE[key] = build_nc(rows_per_core)
    return _NC_CACHE[key]


def kernel(e, A, B):
    from concourse.bass_utils import run_bass_kernel_spmd

    e = np.asarray(e, dtype=np.float32)
    A = np.asarray(A, dtype=np.float32)
    B = np.asarray(B, dtype=np.float32)
    batch, seq, dim = e.shape
    rows = batch * seq
    e16 = np.ascontiguousarray(e.reshape(rows, dim).astype(E_NP))
    # b2[p, k*16+j] = B[k*128+p, j]
    b2 = np.ascontiguousarray(
        B.reshape(dim // 128, 128, RANK).transpose(1, 0, 2).reshape(128, -1).astype(E_NP)
    )
    at = np.ascontiguousarray(A.T.astype(E_NP))

    rpc = rows // N_CORES
    in_maps = [
        {
            "e_in": np.ascontiguousarray(e16[i * rpc : (i + 1) * rpc]),
            "b_in": b2,
            "at_in": at,
        }
        for i in range(N_CORES)
    ]
    nc = _get_nc(rpc)
    res = run_bass_kernel_spmd(nc, in_maps, core_ids=list(range(N_CORES)))
    out = np.concatenate([res.results[i]["out_d"] for i in range(N_CORES)], axis=0)
    return out.reshape(batch, seq, dim).astype(np.float32)
